# revision 28
# baseline (speedup 1.0000x reference)
"""Bass/Trainium2 kernel for BiGraphContrastLayer (GNN message passing).

Computes, for two edge lists (pos/neg) over the same node features:
    h_g = PReLU( D_in^-1/2 A_g D_out^-1/2 feats @ W + b )
returning stack([h_pos, h_neg]) of shape [2, N, Dout].

Strategy (8 NeuronCores, SPMD, no collectives):

Because row-scaling commutes with the right-multiply by W,
    h = nd . ((segsum(ns[src] * feats[src])) @ W),
we gather FEATS rows directly (no feats@W pre-pass at all), fold the
per-src norm ns into the one-hot segment-sum matrix, aggregate
transposed (aggT[d, i] = sum_slot gt[slot, d] * oh[slot, i], i.e.
matmul(lhsT=gathered_chunk, rhs=onehot)), then apply @W per dst tile
and PReLU with the dst norm as the activation scale.

Work split: dst tiles are dealt to the 8 cores (sorted by edge count so
the shared instruction stream is near-optimal for every core). Edges
are bucketed (dst_tile, src_bank) and packed tightly into 128-slot
gather chunks: slot counts are padded only to the cross-core max per
(tile, bank) -- not to a multiple of 128 -- and the dma_gather calls
use exact num_idxs. Chunks spanning two dst tiles are consumed by both
tiles' one-hot matmuls (the one-hot zeroes foreign edges via a
supergroup-relative offset encoding).

Host does integer index work only (sorting, bucketing, degree
bincounts, int16 gather indices); norms are computed on device from
gathered integer degrees.
"""

import hashlib
import math
import pickle
import tempfile
from dataclasses import dataclass

import numpy as np

P = 128   # partitions
D = 128   # feature dim (Din == Dout == 128)
NBANK = 4  # int16 gather indices -> <=32768 rows per gather window
SENT = 512.0  # one-hot sentinel: never matches iota (0..127)


def _cdiv(a, b):
    return -(-a // b)


# --------------------------------------------------------------------------
# Config
# --------------------------------------------------------------------------
@dataclass
class Config:
    n_nodes: int = 100000
    n_cores: int = 8
    sg: int = 10       # dst-tile positions per supergroup (job)
    wbatch: int = 4    # dst tiles per PSUM bank / W-matmul batch
    gbufs: int = 2     # gather buffer count
    ipb: int = 3       # idx buffer count
    ohb: int = 8       # one-hot buffer count
    act_prelu: bool = True   # ScalarE Prelu (not implemented in CoreSim)
    xbatch: int = 8    # unused (kept for test harness compat)

    @property
    def t_global(self) -> int:
        return math.ceil(self.n_nodes / P)

    @property
    def n_pad(self) -> int:
        return self.t_global * P

    @property
    def t_core(self) -> int:
        return math.ceil(self.t_global / self.n_cores)

    @property
    def bank_rows(self) -> int:
        br = _cdiv(_cdiv(self.n_pad, NBANK), P) * P
        assert br <= 32768
        return br

    @property
    def n_pad2(self) -> int:
        return self.bank_rows * NBANK


# --------------------------------------------------------------------------
# Host-side preprocessing (integer index manipulation only)
# --------------------------------------------------------------------------
def _plan_graph(src, dst, cfg: Config):
    """Bucket edges by (dst tile, src bank); deal tiles to cores."""
    tg, ncores, tcore = cfg.t_global, cfg.n_cores, cfg.t_core
    BR = cfg.bank_rows
    src = np.asarray(src, np.int64)
    dst = np.asarray(dst, np.int64)
    deg_out = np.bincount(src, minlength=cfg.n_nodes)
    deg_in = np.bincount(dst, minlength=cfg.n_nodes)

    t = dst // P
    bk = src // BR
    key = t * NBANK + bk
    order = np.argsort(key, kind="stable")
    loc_s = (src - bk * BR)[order].astype(np.int16)
    off_s = (dst[order] % P).astype(np.float32)
    deg_s = deg_out[src[order]].astype(np.int32)
    cnt = np.bincount(key, minlength=tg * NBANK).reshape(tg, NBANK)
    starts = np.zeros(tg * NBANK + 1, np.int64)
    np.cumsum(cnt.reshape(-1), out=starts[1:])

    # Deal tiles to cores: sort by total count desc so each slot's group
    # of n_cores tiles has near-equal size (shared instruction stream),
    # then hill-climb swaps between nearby groups to reduce the
    # per-(group, bank) max (which pads every core's gather).
    tot = cnt.sum(1)
    ordt = np.argsort(-tot, kind="stable")
    nslots = ncores * tcore
    dealt = np.full(nslots, -1, np.int64)
    dealt[:tg] = ordt
    groups = dealt.reshape(tcore, ncores)  # [slot, core] tile ids
    _refine_deal(groups, cnt, cfg)
    _order_slots(groups, cnt, cfg)
    core_tiles = groups.T.copy()  # [ncores, tcore]

    # Shared per-(slot, bank) counts = max over the slot's group.
    C = np.zeros((tcore, NBANK), np.int64)
    for k in range(tcore):
        grp = core_tiles[:, k]
        grp = grp[grp >= 0]
        if len(grp):
            C[k] = cnt[grp].max(0)
        if C[k].sum() == 0:
            C[k, 0] = 1  # keep PSUM chain non-empty for null slots
    return dict(core_tiles=core_tiles, C=C, cnt=cnt, starts=starts,
                loc_s=loc_s, off_s=off_s, deg_s=deg_s,
                deg_out=deg_out, deg_in=deg_in)


def _refine_deal(groups, cnt, cfg: Config, dist=3, sweeps=3):
    """Swap tiles between nearby slot-groups to reduce sum of per-bank
    maxima (= shared gather padding). groups: [t_core, n_cores] in/out."""
    tcore, ncores = groups.shape
    zero = np.zeros(NBANK, np.int64)

    def bank(t):
        return cnt[t] if t >= 0 else zero

    G = np.stack([np.stack([bank(t) for t in groups[k]]) for k in range(tcore)])
    # G: [tcore, ncores, NBANK]
    for _ in range(sweeps):
        improved = False
        for d in range(1, dist + 1):
            for k in range(tcore - d):
                k2 = k + d
                A, B = G[k], G[k2]
                base = A.max(0).sum() + B.max(0).sum()
                best = (0, -1, -1)
                for i in range(ncores):
                    Ai = np.delete(A, i, axis=0)
                    Am = Ai.max(0)
                    for j in range(ncores):
                        Bj = np.delete(B, j, axis=0)
                        c = (np.maximum(Am, B[j]).sum()
                             + np.maximum(Bj.max(0), A[i]).sum())
                        delta = c - base
                        if delta < best[0]:
                            best = (delta, i, j)
                if best[1] >= 0:
                    i, j = best[1], best[2]
                    groups[k, i], groups[k2, j] = groups[k2, j], groups[k, i]
                    G[k, i], G[k2, j] = G[k2, j].copy(), G[k, i].copy()
                    improved = True
        if not improved:
            break


def _order_slots(groups, cnt, cfg: Config):
    """Permute slot order so each supergroup gets a balanced mix of big
    and small tile-groups (keeps the gather DMA stream dense throughout)
    and the final supergroup gets the smallest ones (short drain tail)."""
    tcore = groups.shape[0]
    sizes = np.zeros(tcore, np.int64)
    for k in range(tcore):
        grp = groups[k]
        grp = grp[grp >= 0]
        if len(grp):
            sizes[k] = cnt[grp].max(0).sum()
    njobs = _cdiv(tcore, cfg.sg)
    order = np.argsort(-sizes, kind="stable")
    if njobs <= 1:
        return
    last_kn = tcore - (njobs - 1) * cfg.sg
    head, tail = order[:tcore - last_kn], order[tcore - last_kn:]
    nj = njobs - 1
    perm = []
    for j in range(nj):
        perm.extend(head[j::nj])
    perm.extend(tail)
    groups[:] = groups[np.array(perm)]


def _layout_graph(plan, cfg: Config):
    """Shared job layout: per supergroup, per bank, slot space and the
    per-tile (column, chunk) consumption lists. A chunk spanning two
    tiles gets one masked one-hot column per tile (masking is baked
    into the host-built off values)."""
    C = plan["C"]
    tcore, sg = cfg.t_core, cfg.sg
    jobs = []
    k0 = 0
    while k0 < tcore:
        kn = min(sg, tcore - k0)
        Cj = C[k0:k0 + kn]                       # [kn, NBANK]
        L = Cj.sum(0)                            # slots per bank
        nch = -(-L // P)                         # chunks per bank
        lo = np.zeros(NBANK + 1, np.int64)
        np.cumsum(nch, out=lo[1:])
        pfx = np.zeros((kn + 1, NBANK), np.int64)
        np.cumsum(Cj, axis=0, out=pfx[1:])
        tiles = []                               # per ki: [(col, chunk)]
        col = 0
        for ki in range(kn):
            segs = []
            for b in range(NBANK):
                if Cj[ki, b] == 0:
                    continue
                r0 = int(lo[b] + pfx[ki, b] // P)
                r1 = int(lo[b] + _cdiv(int(pfx[ki, b] + Cj[ki, b]), P))
                for c in range(r0, r1):
                    segs.append((col, c))
                    col += 1
            tiles.append(segs)
        jobs.append(dict(k0=int(k0), kn=int(kn),
                         L=[int(x) for x in L],
                         nch=[int(x) for x in nch],
                         lo=[int(x) for x in lo[:NBANK]],
                         nch_tot=int(lo[NBANK]),
                         tiles=tiles, rcols=int(col)))
        k0 += kn
    return jobs


def _fill_core_graph(plan, jobs, core, cfg: Config):
    """Per-core idx ([128, cols] int16, x8-replicated wrap) and masked
    per-column off/deg ([128, 2*rcols] bf16) arrays, job-after-job."""
    import ml_dtypes
    bf16 = ml_dtypes.bfloat16
    core_t = plan["core_tiles"][core]
    C, cnt, starts = plan["C"], plan["cnt"], plan["starts"]
    loc_s, off_s, deg_s = plan["loc_s"], plan["off_s"], plan["deg_s"]
    idx_blocks, od_blocks = [], []
    for job in jobs:
        k0, kn, nch_tot = job["k0"], job["kn"], job["nch_tot"]
        nslot = nch_tot * P
        idx_arr = np.zeros(nslot, np.int16)
        own = np.full(nslot, -1, np.int64)
        offv = np.full(nslot, SENT, np.float32)
        degv = np.ones(nslot, np.float32)
        for b in range(NBANK):
            pos = job["lo"][b] * P
            for ki in range(kn):
                cki = int(C[k0 + ki, b])
                if cki == 0:
                    continue
                t = int(core_t[k0 + ki])
                ne = int(cnt[t, b]) if t >= 0 else 0
                if ne:
                    s0 = int(starts[t * NBANK + b])
                    idx_arr[pos:pos + ne] = loc_s[s0:s0 + ne]
                    offv[pos:pos + ne] = off_s[s0:s0 + ne]
                    degv[pos:pos + ne] = deg_s[s0:s0 + ne]
                own[pos:pos + cki] = k0 + ki
                pos += cki
        # wrap [i%16, i//16], replicated to all 128 partitions (the 8 Q7
        # cores each read their own 16-partition stripe on real HW)
        idx_blocks.append(np.tile(idx_arr.reshape(-1, 16).T, (8, 1)))
        rc = job["rcols"]
        offc = np.full((P, rc), SENT, np.float32)
        degc = np.ones((P, rc), np.float32)
        o2 = offv.reshape(nch_tot, P)
        d2 = degv.reshape(nch_tot, P)
        w2 = own.reshape(nch_tot, P)
        for ki in range(kn):
            for (col, c) in job["tiles"][ki]:
                m = w2[c] == (k0 + ki)
                offc[:, col] = np.where(m, o2[c], SENT)
                degc[:, col] = np.where(m, d2[c], 1.0)
        od_blocks.append(np.concatenate([offc, degc], axis=1).astype(bf16))
    return np.concatenate(idx_blocks, axis=1), np.concatenate(od_blocks, axis=1)


def preprocess(feats, W, b, prelu_a, src_pos, dst_pos, src_neg, dst_neg,
               cfg: Config):
    import ml_dtypes
    bf16 = ml_dtypes.bfloat16
    n, ncores, tcore = cfg.n_nodes, cfg.n_cores, cfg.t_core
    feats = np.asarray(feats, np.float32)
    W = np.asarray(W, np.float32)
    b = np.asarray(b, np.float32)
    prelu_a = np.asarray(prelu_a, np.float32)

    featsb = np.zeros((cfg.n_pad2, D), bf16)
    featsb[:n] = feats.astype(bf16)

    plans, layouts = [], []
    for src, dst in ((src_pos, dst_pos), (src_neg, dst_neg)):
        plan = _plan_graph(src, dst, cfg)
        plans.append(plan)
        layouts.append(_layout_graph(plan, cfg))

    # interleave the two graphs' jobs
    jobs_flat = []
    for si in range(max(len(layouts[0]), len(layouts[1]))):
        for g in range(2):
            if si < len(layouts[g]):
                jobs_flat.append((g, si))

    degi_arr = np.zeros((ncores, P, 2 * tcore), np.int32)
    for g in range(2):
        dpad = np.zeros(cfg.n_pad, np.int32)
        dpad[:n] = plans[g]["deg_in"]
        dpad_t = dpad.reshape(cfg.t_global, P).T
        for core in range(ncores):
            ct = plans[g]["core_tiles"][core]
            valid = ct >= 0
            degi_arr[core, :, g * tcore:(g + 1) * tcore][:, valid] = (
                dpad_t[:, ct[valid]])

    a_rep = np.full((P, 1), float(prelu_a.reshape(-1)[0]), np.float32)
    w_b = W.astype(bf16)
    b_rep = np.tile(b.reshape(1, D), (P, 1)).astype(np.float32)

    # iota_t[p, i*rc_max + c] = i  (transposed one-hot compare constant)
    rc_max = max(j["rcols"] for jobs in layouts for j in jobs)
    iota_t = np.tile(
        np.repeat(np.arange(P, dtype=np.float32), rc_max)[None, :],
        (P, 1)).astype(bf16)

    in_maps = []
    for core in range(ncores):
        iw = [None, None]
        od = [None, None]
        for g in range(2):
            iw[g], od[g] = _fill_core_graph(plans[g], layouts[g], core, cfg)
        # assemble in jobs_flat order
        icols = [[0], [0]]
        ocols = [[0], [0]]
        for g in range(2):
            for job in layouts[g]:
                icols[g].append(icols[g][-1] + job["nch_tot"] * 8)
                ocols[g].append(ocols[g][-1] + 2 * job["rcols"])
        idx_parts, od_parts = [], []
        for (g, si) in jobs_flat:
            idx_parts.append(iw[g][:, icols[g][si]:icols[g][si + 1]])
            od_parts.append(od[g][:, ocols[g][si]:ocols[g][si + 1]])
        in_maps.append({
            "featsb": featsb,
            "w_in": w_b,
            "a_rep": a_rep,
            "b_rep": b_rep,
            "degi": degi_arr[core],
            "idx_in": np.ascontiguousarray(np.concatenate(idx_parts, axis=1)),
            "od_in": np.ascontiguousarray(np.concatenate(od_parts, axis=1)),
            "iota_t": iota_t,
        })
    meta = {
        "layouts": layouts,
        "jobs_flat": jobs_flat,
        "use_bias": bool(np.any(b != 0.0)),
    }
    return in_maps, plans, meta


# --------------------------------------------------------------------------
# Device kernel builder
# --------------------------------------------------------------------------
def build_kernel(nc, tc, cfg: Config, meta):
    from contextlib import ExitStack

    import concourse.mybir as mybir

    f32 = mybir.dt.float32
    bf16 = mybir.dt.bfloat16
    i32 = mybir.dt.int32
    i16 = mybir.dt.int16
    Alu = mybir.AluOpType
    Act = mybir.ActivationFunctionType

    tcore, BR = cfg.t_core, cfg.bank_rows
    layouts = meta["layouts"]
    jobs_flat = meta["jobs_flat"]
    use_bias = meta["use_bias"]

    jobs = [layouts[g][si] for (g, si) in jobs_flat]
    icols_tot = sum(j["nch_tot"] * 8 for j in jobs)
    odcols_tot = sum(2 * j["rcols"] for j in jobs)
    nch_max = max(j["nch_tot"] for j in jobs)
    rc_max = max(j["rcols"] for j in jobs)

    featsb = nc.dram_tensor("featsb", [cfg.n_pad2, D], bf16,
                            kind="ExternalInput").ap()
    w_in = nc.dram_tensor("w_in", [P, D], bf16, kind="ExternalInput").ap()
    a_rep = nc.dram_tensor("a_rep", [P, 1], f32, kind="ExternalInput").ap()
    b_rep = nc.dram_tensor("b_rep", [P, D], f32, kind="ExternalInput").ap()
    degi = nc.dram_tensor("degi", [P, 2 * tcore], i32, kind="ExternalInput").ap()
    idx_in = nc.dram_tensor("idx_in", [P, icols_tot], i16,
                            kind="ExternalInput").ap()
    od_in = nc.dram_tensor("od_in", [P, odcols_tot], bf16,
                           kind="ExternalInput").ap()
    iota_t = nc.dram_tensor("iota_t", [P, P * rc_max], bf16,
                            kind="ExternalInput").ap()
    out = nc.dram_tensor("out", [2, P, tcore, D], bf16,
                         kind="ExternalOutput").ap()

    with ExitStack() as ctx:
        const = ctx.enter_context(tc.tile_pool(name="const", bufs=1))
        work = ctx.enter_context(tc.tile_pool(name="work", bufs=2))
        ipool = ctx.enter_context(tc.tile_pool(name="ipool", bufs=cfg.ipb))
        odpool = ctx.enter_context(tc.tile_pool(name="odpool", bufs=3))
        nspool = ctx.enter_context(tc.tile_pool(name="nspool", bufs=3))
        gpool = ctx.enter_context(tc.tile_pool(name="gpool", bufs=cfg.gbufs))
        eqpool = ctx.enter_context(tc.tile_pool(name="eqpool", bufs=1))
        ohpool = ctx.enter_context(tc.tile_pool(name="ohpool", bufs=2))
        apool = ctx.enter_context(tc.tile_pool(name="apool", bufs=3))
        stpool = ctx.enter_context(tc.tile_pool(name="stpool", bufs=2))
        tpool = ctx.enter_context(tc.tile_pool(name="tpool", bufs=4))
        ppool = ctx.enter_context(tc.tile_pool(name="ppool", bufs=3,
                                               space="PSUM"))
        hpool = ctx.enter_context(tc.tile_pool(name="hpool", bufs=4,
                                               space="PSUM"))

        # ---- constants ----
        w_sb = const.tile([P, D], bf16)
        nc.sync.dma_start(out=w_sb[:], in_=w_in)
        iota_sb = const.tile([P, P * rc_max], bf16)
        nc.sync.dma_start(out=iota_sb[:], in_=iota_t)
        a_sb = const.tile([P, 1], f32)
        nc.sync.dma_start(out=a_sb[:], in_=a_rep)
        if use_bias:
            b_sb = const.tile([P, D], f32)
            nc.sync.dma_start(out=b_sb[:], in_=b_rep)

        # ---- dst norms from in-degrees: nd = (deg>0) / sqrt(max(deg,1)) ----
        width = 2 * tcore
        dg = work.tile([P, width], i32, tag="dg")
        nc.sync.dma_start(out=dg[:], in_=degi)
        f = work.tile([P, width], f32, tag="f")
        nc.vector.tensor_copy(out=f[:], in_=dg[:])
        m = work.tile([P, width], f32, tag="m")
        nc.vector.tensor_scalar(out=m[:], in0=f[:], scalar1=1.0,
                                scalar2=None, op0=Alu.max)
        r = work.tile([P, width], f32, tag="r")
        nc.vector.reciprocal(out=r[:], in_=m[:])
        s = work.tile([P, width], f32, tag="s")
        nc.scalar.activation(out=s[:], in_=r[:], func=Act.Sqrt)
        z = work.tile([P, width], f32, tag="z")
        nc.vector.tensor_scalar(out=z[:], in0=f[:], scalar1=1.0,
                                scalar2=None, op0=Alu.min)
        nd_sb = const.tile([P, width], f32)
        nc.vector.tensor_tensor(out=nd_sb[:], in0=s[:], in1=z[:], op=Alu.mult)
        and_sb = const.tile([P, width], f32)
        nc.vector.tensor_tensor(out=and_sb[:], in0=nd_sb[:],
                                in1=a_sb[:, :1].to_broadcast([P, width]),
                                op=Alu.mult)

        ic0 = 0
        oc0 = 0
        for jidx, (g, si) in enumerate(jobs_flat):
            job = layouts[g][si]
            k0, kn = job["k0"], job["kn"]
            ncht, rc = job["nch_tot"], job["rcols"]
            icols = ncht * 8

            it = ipool.tile([P, nch_max * 8], i16, tag="gidx")
            nc.sync.dma_start(out=it[:, 0:icols],
                              in_=idx_in[:, ic0:ic0 + icols])
            od = odpool.tile([P, 2 * rc_max], bf16, tag="offdeg")
            nc.sync.dma_start(out=od[:, 0:2 * rc],
                              in_=od_in[:, oc0:oc0 + 2 * rc])
            # per-column src norm: ns = 1/sqrt(deg)  (deg>=1 by construction)
            nsr = nspool.tile([P, rc_max], f32, tag="nsr")
            nc.vector.reciprocal(out=nsr[:, :rc], in_=od[:, rc:2 * rc])
            nse = nspool.tile([P, rc_max], bf16, tag="nse")
            nc.scalar.activation(out=nse[:, :rc], in_=nsr[:, :rc],
                                 func=Act.Sqrt)

            gt = gpool.tile([P, nch_max, D], bf16, tag="gather")
            for b in range(NBANK):
                L = job["L"][b]
                if L == 0:
                    continue
                nchb = job["nch"][b]
                lob = job["lo"][b]
                ni = nchb * P  # full chunks: every read byte gets written
                nc.gpsimd.dma_gather(
                    out_ap=gt[:, lob:lob + nchb, :],
                    in_ap=featsb[b * BR:(b + 1) * BR, :],
                    idxs_ap=it[:, lob * 8:lob * 8 + ni // 16],
                    num_idxs=ni, num_idxs_reg=ni,
                    elem_size=D, single_packet=False)

            # scaled one-hots for ALL columns in two big DVE ops:
            #   ohs[p, i, col] = (iota[i] == off[p, col]) * ns[p, col]
            eq = eqpool.tile([P, P, rc_max], bf16, tag="eq")
            off_b = od[:, 0:rc].rearrange(
                "p (c o) -> p o c", o=1).to_broadcast([P, P, rc])
            iot_b = iota_sb[:].rearrange("p (i c) -> p i c", c=rc_max)
            nc.vector.tensor_tensor(out=eq[:, :, :rc],
                                    in0=iot_b[:, :, :rc],
                                    in1=off_b, op=Alu.is_equal)
            ohs = ohpool.tile([P, P, rc_max], bf16, tag="ohs")
            ns_b = nse[:, :rc].rearrange(
                "p (c o) -> p o c", o=1).to_broadcast([P, P, rc])
            nc.vector.tensor_tensor(out=ohs[:, :, :rc], in0=eq[:, :, :rc],
                                    in1=ns_b, op=Alu.mult)

            stg = stpool.tile([P, cfg.sg, D], bf16, tag="stg")
            ki = 0
            while ki < kn:
                bn = min(cfg.wbatch, kn - ki)
                ap_ = ppool.tile([P, cfg.wbatch, D], f32)
                for j2 in range(bn):
                    segs = job["tiles"][ki + j2]
                    for si2, (col, c) in enumerate(segs):
                        nc.tensor.matmul(
                            out=ap_[:, j2, :],
                            lhsT=gt[:, c, :],
                            rhs=ohs[:, :, col:col + 1].rearrange(
                                "p i o -> p (i o)"),
                            start=(si2 == 0),
                            stop=(si2 == len(segs) - 1))
                asb = apool.tile([P, cfg.wbatch, D], bf16, tag="aggsb")
                nc.scalar.activation(out=asb[:, :bn, :], in_=ap_[:, :bn, :],
                                     func=Act.Copy)
                for j2 in range(bn):
                    h = hpool.tile([P, D], f32)
                    nc.tensor.matmul(out=h[:], lhsT=asb[:, j2, :], rhs=w_sb[:],
                                     start=True, stop=True)
                    kslot = g * tcore + k0 + ki + j2
                    if cfg.act_prelu and not use_bias:
                        nc.scalar.activation(
                            out=stg[:, ki + j2, :], in_=h[:], func=Act.Prelu,
                            scale=nd_sb[:, kslot:kslot + 1], alpha=a_sb[:, :1])
                        continue
                    if use_bias:
                        hb = tpool.tile([P, D], f32, tag="hb")
                        nc.vector.tensor_scalar(
                            out=hb[:], in0=h[:],
                            scalar1=nd_sb[:, kslot:kslot + 1],
                            scalar2=None, op0=Alu.mult)
                        hb2 = tpool.tile([P, D], f32, tag="hb2")
                        nc.vector.tensor_tensor(out=hb2[:], in0=hb[:],
                                                in1=b_sb[:], op=Alu.add)
                        neg = tpool.tile([P, D], f32, tag="neg")
                        nc.vector.tensor_scalar(
                            out=neg[:], in0=hb2[:], scalar1=0.0,
                            scalar2=a_sb[:, :1], op0=Alu.min, op1=Alu.mult)
                        pos = tpool.tile([P, D], f32, tag="pos")
                        nc.vector.tensor_scalar(
                            out=pos[:], in0=hb2[:], scalar1=0.0,
                            scalar2=None, op0=Alu.max)
                    else:
                        neg = tpool.tile([P, D], f32, tag="neg")
                        nc.vector.tensor_scalar(
                            out=neg[:], in0=h[:], scalar1=0.0,
                            scalar2=and_sb[:, kslot:kslot + 1],
                            op0=Alu.min, op1=Alu.mult)
                        pos = tpool.tile([P, D], f32, tag="pos")
                        nc.vector.tensor_scalar(
                            out=pos[:], in0=h[:], scalar1=0.0,
                            scalar2=nd_sb[:, kslot:kslot + 1],
                            op0=Alu.max, op1=Alu.mult)
                    nc.vector.tensor_tensor(out=stg[:, ki + j2, :], in0=neg[:],
                                            in1=pos[:], op=Alu.add)
                ki += bn
            nc.sync.dma_start(out=out[g, :, k0:k0 + kn, :], in_=stg[:, :kn, :])
            ic0 += icols
            oc0 += 2 * rc
    return out


# --------------------------------------------------------------------------
# Driver
# --------------------------------------------------------------------------
def _build_program(cfg: Config, meta):
    import concourse.bacc as bacc
    import concourse.tile as tile

    nc = bacc.Bacc("TRN2", target_bir_lowering=False, debug=False,
                   enable_asserts=False, num_devices=cfg.n_cores)
    with tile.TileContext(nc) as tc:
        build_kernel(nc, tc, cfg, meta)
    nc.compile()
    return nc


def _unscramble(results, plans, cfg: Config):
    n = cfg.n_nodes
    full = np.zeros((2, n, D), np.float32)
    for g in range(2):
        ct_all = plans[g]["core_tiles"]
        for core in range(cfg.n_cores):
            oc = np.asarray(results[core]["out"], dtype=np.float32)
            # oc: [2, P, t_core, D]
            for k in range(cfg.t_core):
                t = int(ct_all[core, k])
                if t < 0:
                    continue
                r0 = t * P
                r1 = min(r0 + P, n)
                full[g, r0:r1] = oc[g, :r1 - r0, k, :]
    return full


_PROGRAM_CACHE = {}


def _meta_key(cfg: Config, meta):
    sig = (cfg.n_nodes, cfg.n_cores, cfg.sg, cfg.wbatch, cfg.gbufs, cfg.ipb,
           cfg.ohb, cfg.act_prelu, meta["use_bias"], meta["jobs_flat"],
           meta["layouts"])
    return hashlib.md5(pickle.dumps(sig)).hexdigest()


def run(inputs, cfg: Config, trace=False):
    from concourse.bass_utils import run_bass_kernel_spmd

    in_maps, plans, meta = preprocess(
        inputs["feats"], inputs["W"], inputs["b"], inputs["prelu_a"],
        inputs["src_pos"], inputs["dst_pos"],
        inputs["src_neg"], inputs["dst_neg"], cfg)

    key = _meta_key(cfg, meta)
    nc = _PROGRAM_CACHE.get(key)
    if nc is None:
        nc = _build_program(cfg, meta)
        _PROGRAM_CACHE[key] = nc

    kwargs = {}
    if trace:
        kwargs = dict(trace=True, tmpdir=tempfile.mkdtemp(prefix="bgc_trace_"))
    res = run_bass_kernel_spmd(nc, in_maps, core_ids=list(range(cfg.n_cores)),
                               **kwargs)
    full = _unscramble(res.results, plans, cfg)
    return full, res


def kernel(**inputs) -> np.ndarray:
    cfg = Config()
    full, _ = run(inputs, cfg)
    return full


# revision 37
# speedup vs baseline: 1.1132x; 1.1132x over previous
"""Bass/Trainium2 kernel for BiGraphContrastLayer (GNN message passing).

Computes, for two edge lists (pos/neg) over the same node features:
    h_g = PReLU( D_in^-1/2 A_g D_out^-1/2 feats @ W + b )
returning stack([h_pos, h_neg]) of shape [2, N, Dout].

Strategy (8 NeuronCores, SPMD, no collectives):

Because row-scaling commutes with the right-multiply by W,
    h = nd . ((segsum(ns[src] * feats[src])) @ W),
we gather FEATS rows directly (no feats@W pre-pass at all), fold the
per-src norm ns into the one-hot segment-sum matrix, aggregate
transposed (aggT[d, i] = sum_slot gt[slot, d] * oh[slot, i], i.e.
matmul(lhsT=gathered_chunk, rhs=onehot)), then apply @W per dst tile
and PReLU with the dst norm as the activation scale.

Work split: dst tiles are dealt to the 8 cores (sorted by edge count so
the shared instruction stream is near-optimal for every core). Edges
are bucketed (dst_tile, src_bank) and packed tightly into 128-slot
gather chunks: slot counts are padded only to the cross-core max per
(tile, bank) -- not to a multiple of 128 -- and the dma_gather calls
use exact num_idxs. Chunks spanning two dst tiles are consumed by both
tiles' one-hot matmuls (the one-hot zeroes foreign edges via a
supergroup-relative offset encoding).

Host does integer index work only (sorting, bucketing, degree
bincounts, int16 gather indices); norms are computed on device from
gathered integer degrees.
"""

import hashlib
import math
import pickle
import tempfile
from dataclasses import dataclass

import numpy as np

P = 128   # partitions
D = 128   # feature dim (Din == Dout == 128)
NBANK = 4  # int16 gather indices -> <=32768 rows per gather window
SENT = 512.0  # one-hot sentinel: never matches iota (0..127)


def _cdiv(a, b):
    return -(-a // b)


# --------------------------------------------------------------------------
# Config
# --------------------------------------------------------------------------
@dataclass
class Config:
    n_nodes: int = 100000
    n_cores: int = 8
    sg: int = 10       # dst-tile positions per supergroup (job)
    wbatch: int = 4    # dst tiles per PSUM bank / W-matmul batch
    gbufs: int = 2     # gather buffer count
    ipb: int = 3       # idx buffer count
    ohb: int = 8       # one-hot buffer count
    act_prelu: bool = True   # ScalarE Prelu (not implemented in CoreSim)
    xbatch: int = 8    # unused (kept for test harness compat)

    @property
    def t_global(self) -> int:
        return math.ceil(self.n_nodes / P)

    @property
    def n_pad(self) -> int:
        return self.t_global * P

    @property
    def t_core(self) -> int:
        return math.ceil(self.t_global / self.n_cores)

    @property
    def bank_rows(self) -> int:
        br = _cdiv(_cdiv(self.n_pad, NBANK), P) * P
        assert br <= 32768
        return br

    @property
    def n_pad2(self) -> int:
        return self.bank_rows * NBANK


# --------------------------------------------------------------------------
# Host-side preprocessing (integer index manipulation only)
# --------------------------------------------------------------------------
def _plan_graph(src, dst, cfg: Config):
    """Bucket edges by (dst tile, src bank); deal tiles to cores."""
    tg, ncores, tcore = cfg.t_global, cfg.n_cores, cfg.t_core
    BR = cfg.bank_rows
    src = np.asarray(src, np.int64)
    dst = np.asarray(dst, np.int64)
    deg_out = np.bincount(src, minlength=cfg.n_nodes)
    deg_in = np.bincount(dst, minlength=cfg.n_nodes)

    t = dst // P
    bk = src // BR
    key = t * NBANK + bk
    order = np.argsort(key, kind="stable")
    loc_s = (src - bk * BR)[order].astype(np.int16)
    off_s = (dst[order] % P).astype(np.float32)
    deg_s = deg_out[src[order]].astype(np.int32)
    cnt = np.bincount(key, minlength=tg * NBANK).reshape(tg, NBANK)
    starts = np.zeros(tg * NBANK + 1, np.int64)
    np.cumsum(cnt.reshape(-1), out=starts[1:])

    # Deal tiles to cores: sort by total count desc so each slot's group
    # of n_cores tiles has near-equal size (shared instruction stream),
    # then hill-climb swaps between nearby groups to reduce the
    # per-(group, bank) max (which pads every core's gather).
    tot = cnt.sum(1)
    ordt = np.argsort(-tot, kind="stable")
    nslots = ncores * tcore
    dealt = np.full(nslots, -1, np.int64)
    dealt[:tg] = ordt
    groups = dealt.reshape(tcore, ncores)  # [slot, core] tile ids
    _refine_deal(groups, cnt, cfg)
    _order_slots(groups, cnt, cfg)
    core_tiles = groups.T.copy()  # [ncores, tcore]

    # Shared per-(slot, bank) counts = max over the slot's group.
    C = np.zeros((tcore, NBANK), np.int64)
    for k in range(tcore):
        grp = core_tiles[:, k]
        grp = grp[grp >= 0]
        if len(grp):
            C[k] = cnt[grp].max(0)
        if C[k].sum() == 0:
            C[k, 0] = 1  # keep PSUM chain non-empty for null slots
    return dict(core_tiles=core_tiles, C=C, cnt=cnt, starts=starts,
                loc_s=loc_s, off_s=off_s, deg_s=deg_s,
                deg_out=deg_out, deg_in=deg_in)


def _refine_deal(groups, cnt, cfg: Config, dist=3, sweeps=3):
    """Swap tiles between nearby slot-groups to reduce sum of per-bank
    maxima (= shared gather padding). groups: [t_core, n_cores] in/out."""
    tcore, ncores = groups.shape
    zero = np.zeros(NBANK, np.int64)

    def bank(t):
        return cnt[t] if t >= 0 else zero

    G = np.stack([np.stack([bank(t) for t in groups[k]]) for k in range(tcore)])
    # G: [tcore, ncores, NBANK]
    for _ in range(sweeps):
        improved = False
        for d in range(1, dist + 1):
            for k in range(tcore - d):
                k2 = k + d
                A, B = G[k], G[k2]
                base = A.max(0).sum() + B.max(0).sum()
                best = (0, -1, -1)
                for i in range(ncores):
                    Ai = np.delete(A, i, axis=0)
                    Am = Ai.max(0)
                    for j in range(ncores):
                        Bj = np.delete(B, j, axis=0)
                        c = (np.maximum(Am, B[j]).sum()
                             + np.maximum(Bj.max(0), A[i]).sum())
                        delta = c - base
                        if delta < best[0]:
                            best = (delta, i, j)
                if best[1] >= 0:
                    i, j = best[1], best[2]
                    groups[k, i], groups[k2, j] = groups[k2, j], groups[k, i]
                    G[k, i], G[k2, j] = G[k2, j].copy(), G[k, i].copy()
                    improved = True
        if not improved:
            break


def _order_slots(groups, cnt, cfg: Config):
    """Permute slot order so each supergroup gets a balanced mix of big
    and small tile-groups (keeps the gather DMA stream dense throughout)
    and the final supergroup gets the smallest ones (short drain tail)."""
    tcore = groups.shape[0]
    sizes = np.zeros(tcore, np.int64)
    for k in range(tcore):
        grp = groups[k]
        grp = grp[grp >= 0]
        if len(grp):
            sizes[k] = cnt[grp].max(0).sum()
    njobs = _cdiv(tcore, cfg.sg)
    order = np.argsort(-sizes, kind="stable")
    if njobs <= 1:
        return
    last_kn = tcore - (njobs - 1) * cfg.sg
    head, tail = order[:tcore - last_kn], order[tcore - last_kn:]
    nj = njobs - 1
    perm = []
    for j in range(nj):
        perm.extend(head[j::nj])
    perm.extend(tail)
    groups[:] = groups[np.array(perm)]


def _layout_graph(plan, cfg: Config):
    """Shared job layout: per supergroup, per bank, slot space and the
    per-tile (column, chunk) consumption lists. A chunk spanning two
    tiles gets one masked one-hot column per tile (masking is baked
    into the host-built off values)."""
    C = plan["C"]
    tcore, sg = cfg.t_core, cfg.sg
    jobs = []
    k0 = 0
    while k0 < tcore:
        kn = min(sg, tcore - k0)
        Cj = C[k0:k0 + kn]                       # [kn, NBANK]
        L = Cj.sum(0)                            # slots per bank
        nch = -(-L // P)                         # chunks per bank
        lo = np.zeros(NBANK + 1, np.int64)
        np.cumsum(nch, out=lo[1:])
        pfx = np.zeros((kn + 1, NBANK), np.int64)
        np.cumsum(Cj, axis=0, out=pfx[1:])
        # per-tile chunk ranges
        ranges = [[] for _ in range(kn)]         # ki -> [chunk, ...]
        for ki in range(kn):
            for b in range(NBANK):
                if Cj[ki, b] == 0:
                    continue
                r0 = int(lo[b] + pfx[ki, b] // P)
                r1 = int(lo[b] + _cdiv(int(pfx[ki, b] + Cj[ki, b]), P))
                ranges[ki].extend(range(r0, r1))
        # Merge: a chunk shared by adjacent tiles in the same wbatch batch
        # gets ONE 256-wide one-hot column + ONE matmul into both tiles'
        # PSUM regions (a leading zero-matmul per batch makes start flags
        # moot). cols: (chunk, ki, ntiles); batches: per batch the ordered
        # matmul plan [(colidx, chunk, j2, nt, stop_kis)].
        wb = cfg.wbatch
        cols = []
        batches = []
        for jb0 in range(0, kn, wb):
            bn = min(wb, kn - jb0)
            merged = []
            excl = {ki: [] for ki in range(jb0, jb0 + bn)}
            handled = set()
            for ki in range(jb0, jb0 + bn):
                nxt = ki + 1
                nxt_in = nxt < jb0 + bn
                for c in ranges[ki]:
                    if (c, ki) in handled:
                        continue
                    if (nxt_in and c in ranges[nxt]
                            and len(ranges[ki]) > 1 and len(ranges[nxt]) > 1):
                        ci = len(cols)
                        cols.append((c, ki, 2))
                        merged.append((ci, c, ki - jb0))
                        handled.add((c, ki))
                        handled.add((c, nxt))
                    else:
                        ci = len(cols)
                        cols.append((c, ki, 1))
                        excl[ki].append((ci, c, ki - jb0))
                        handled.add((c, ki))
            # guard: every tile needs >=1 exclusive matmul (stop carrier)
            for ki in range(jb0, jb0 + bn):
                if not excl[ki]:
                    for mi, (cidx, c, j2) in enumerate(merged):
                        kia, kib = cols[cidx][1], cols[cidx][1] + 1
                        if ki in (kia, kib):
                            cols[cidx] = (c, kia, 1)
                            excl[kia].append((cidx, c, kia - jb0))
                            ci2 = len(cols)
                            cols.append((c, kib, 1))
                            excl[kib].append((ci2, c, kib - jb0))
                            merged.pop(mi)
                            break
            plan = []
            for (ci, c, j2) in merged:
                plan.append((ci, c, j2, 2, False))
            for ki in range(jb0, jb0 + bn):
                for n_, (ci, c, j2) in enumerate(excl[ki]):
                    plan.append((ci, c, j2, 1, n_ == len(excl[ki]) - 1))
            batches.append(dict(jb0=jb0, bn=bn, plan=plan))
        jobs.append(dict(k0=int(k0), kn=int(kn),
                         L=[int(x) for x in L],
                         nch=[int(x) for x in nch],
                         lo=[int(x) for x in lo[:NBANK]],
                         nch_tot=int(lo[NBANK]),
                         cols=cols, batches=batches, rcols=len(cols)))
        k0 += kn
    return jobs


def _fill_core_graph(plan, jobs, core, cfg: Config):
    """Per-core idx ([128, cols] int16, x8-replicated wrap) and masked
    per-column off/deg ([128, 2*rcols] bf16) arrays, job-after-job."""
    import ml_dtypes
    bf16 = ml_dtypes.bfloat16
    core_t = plan["core_tiles"][core]
    C, cnt, starts = plan["C"], plan["cnt"], plan["starts"]
    loc_s, off_s, deg_s = plan["loc_s"], plan["off_s"], plan["deg_s"]
    idx_blocks, od_blocks = [], []
    for job in jobs:
        k0, kn, nch_tot = job["k0"], job["kn"], job["nch_tot"]
        nslot = nch_tot * P
        idx_arr = np.zeros(nslot, np.int16)
        own = np.full(nslot, -1, np.int64)
        offv = np.full(nslot, SENT, np.float32)
        degv = np.ones(nslot, np.float32)
        for b in range(NBANK):
            pos = job["lo"][b] * P
            for ki in range(kn):
                cki = int(C[k0 + ki, b])
                if cki == 0:
                    continue
                t = int(core_t[k0 + ki])
                ne = int(cnt[t, b]) if t >= 0 else 0
                if ne:
                    s0 = int(starts[t * NBANK + b])
                    idx_arr[pos:pos + ne] = loc_s[s0:s0 + ne]
                    offv[pos:pos + ne] = off_s[s0:s0 + ne]
                    degv[pos:pos + ne] = deg_s[s0:s0 + ne]
                own[pos:pos + cki] = k0 + ki
                pos += cki
        # wrap [i%16, i//16], replicated to all 128 partitions (the 8 Q7
        # cores each read their own 16-partition stripe on real HW)
        idx_blocks.append(np.tile(idx_arr.reshape(-1, 16).T, (8, 1)))
        rc = job["rcols"]
        offc = np.full((P, rc), SENT, np.float32)
        degc = np.ones((P, rc), np.float32)
        o2 = offv.reshape(nch_tot, P)
        d2 = degv.reshape(nch_tot, P)
        w2 = own.reshape(nch_tot, P)
        for col, (c, ki, nt) in enumerate(job["cols"]):
            m0 = w2[c] == (k0 + ki)
            offc[:, col] = np.where(m0, o2[c], SENT)
            degc[:, col] = np.where(m0, d2[c], 1.0)
            if nt == 2:
                m1 = w2[c] == (k0 + ki + 1)
                offc[:, col] = np.where(m1, o2[c] + P, offc[:, col])
                degc[:, col] = np.where(m1, d2[c], degc[:, col])
        od_blocks.append(np.concatenate([offc, degc], axis=1))
    return np.concatenate(idx_blocks, axis=1), np.concatenate(od_blocks, axis=1)


def preprocess(feats, W, b, prelu_a, src_pos, dst_pos, src_neg, dst_neg,
               cfg: Config):
    import ml_dtypes
    bf16 = ml_dtypes.bfloat16
    n, ncores, tcore = cfg.n_nodes, cfg.n_cores, cfg.t_core
    feats = np.asarray(feats, np.float32)
    W = np.asarray(W, np.float32)
    b = np.asarray(b, np.float32)
    prelu_a = np.asarray(prelu_a, np.float32)

    featsb = np.zeros((cfg.n_pad2, D), bf16)
    featsb[:n] = feats.astype(bf16)

    plans, layouts = [], []
    for src, dst in ((src_pos, dst_pos), (src_neg, dst_neg)):
        plan = _plan_graph(src, dst, cfg)
        plans.append(plan)
        layouts.append(_layout_graph(plan, cfg))

    # interleave the two graphs' jobs
    jobs_flat = []
    for si in range(max(len(layouts[0]), len(layouts[1]))):
        for g in range(2):
            if si < len(layouts[g]):
                jobs_flat.append((g, si))

    degi_arr = np.zeros((ncores, P, 2 * tcore), np.int32)
    for g in range(2):
        dpad = np.zeros(cfg.n_pad, np.int32)
        dpad[:n] = plans[g]["deg_in"]
        dpad_t = dpad.reshape(cfg.t_global, P).T
        for core in range(ncores):
            ct = plans[g]["core_tiles"][core]
            valid = ct >= 0
            degi_arr[core, :, g * tcore:(g + 1) * tcore][:, valid] = (
                dpad_t[:, ct[valid]])

    a_rep = np.full((P, 1), float(prelu_a.reshape(-1)[0]), np.float32)
    w_b = W.astype(bf16)
    b_rep = np.tile(b.reshape(1, D), (P, 1)).astype(np.float32)

    iota_t = np.tile(np.arange(2 * P, dtype=np.float32), (P, 1)).astype(bf16)

    in_maps = []
    for core in range(ncores):
        iw = [None, None]
        od = [None, None]
        for g in range(2):
            iw[g], od[g] = _fill_core_graph(plans[g], layouts[g], core, cfg)
        # assemble in jobs_flat order
        icols = [[0], [0]]
        ocols = [[0], [0]]
        for g in range(2):
            for job in layouts[g]:
                icols[g].append(icols[g][-1] + job["nch_tot"] * 8)
                ocols[g].append(ocols[g][-1] + 2 * job["rcols"])
        idx_parts, od_parts = [], []
        for (g, si) in jobs_flat:
            idx_parts.append(iw[g][:, icols[g][si]:icols[g][si + 1]])
            od_parts.append(od[g][:, ocols[g][si]:ocols[g][si + 1]])
        in_maps.append({
            "featsb": featsb,
            "w_in": w_b,
            "a_rep": a_rep,
            "b_rep": b_rep,
            "degi": degi_arr[core],
            "idx_in": np.ascontiguousarray(np.concatenate(idx_parts, axis=1)),
            "od_in": np.ascontiguousarray(np.concatenate(od_parts, axis=1)),
            "iota_t": iota_t,
        })
    meta = {
        "layouts": layouts,
        "jobs_flat": jobs_flat,
        "use_bias": bool(np.any(b != 0.0)),
    }
    return in_maps, plans, meta


# --------------------------------------------------------------------------
# Device kernel builder
# --------------------------------------------------------------------------
def build_kernel(nc, tc, cfg: Config, meta):
    from contextlib import ExitStack

    import concourse.mybir as mybir

    f32 = mybir.dt.float32
    bf16 = mybir.dt.bfloat16
    i32 = mybir.dt.int32
    i16 = mybir.dt.int16
    Alu = mybir.AluOpType
    Act = mybir.ActivationFunctionType

    tcore, BR = cfg.t_core, cfg.bank_rows
    layouts = meta["layouts"]
    jobs_flat = meta["jobs_flat"]
    use_bias = meta["use_bias"]

    jobs = [layouts[g][si] for (g, si) in jobs_flat]
    icols_tot = sum(j["nch_tot"] * 8 for j in jobs)
    odcols_tot = sum(2 * j["rcols"] for j in jobs)
    nch_max = max(j["nch_tot"] for j in jobs)
    rc_max = max(j["rcols"] for j in jobs)

    featsb = nc.dram_tensor("featsb", [cfg.n_pad2, D], bf16,
                            kind="ExternalInput").ap()
    w_in = nc.dram_tensor("w_in", [P, D], bf16, kind="ExternalInput").ap()
    a_rep = nc.dram_tensor("a_rep", [P, 1], f32, kind="ExternalInput").ap()
    b_rep = nc.dram_tensor("b_rep", [P, D], f32, kind="ExternalInput").ap()
    degi = nc.dram_tensor("degi", [P, 2 * tcore], i32, kind="ExternalInput").ap()
    idx_in = nc.dram_tensor("idx_in", [P, icols_tot], i16,
                            kind="ExternalInput").ap()
    od_in = nc.dram_tensor("od_in", [P, odcols_tot], f32,
                           kind="ExternalInput").ap()
    iota_t = nc.dram_tensor("iota_t", [P, 2 * P], bf16,
                            kind="ExternalInput").ap()
    out = nc.dram_tensor("out", [2, P, tcore, D], bf16,
                         kind="ExternalOutput").ap()

    with ExitStack() as ctx:
        const = ctx.enter_context(tc.tile_pool(name="const", bufs=1))
        work = ctx.enter_context(tc.tile_pool(name="work", bufs=2))
        ipool = ctx.enter_context(tc.tile_pool(name="ipool", bufs=cfg.ipb))
        odpool = ctx.enter_context(tc.tile_pool(name="odpool", bufs=3))
        nspool = ctx.enter_context(tc.tile_pool(name="nspool", bufs=3))
        gpool = ctx.enter_context(tc.tile_pool(name="gpool", bufs=cfg.gbufs))
        ohpool = ctx.enter_context(tc.tile_pool(name="ohpool", bufs=cfg.ohb))
        apool = ctx.enter_context(tc.tile_pool(name="apool", bufs=3))
        stpool = ctx.enter_context(tc.tile_pool(name="stpool", bufs=2))
        tpool = ctx.enter_context(tc.tile_pool(name="tpool", bufs=4))
        ppool = ctx.enter_context(tc.tile_pool(name="ppool", bufs=3,
                                               space="PSUM"))
        hpool = ctx.enter_context(tc.tile_pool(name="hpool", bufs=4,
                                               space="PSUM"))

        # ---- constants ----
        w_sb = const.tile([P, D], bf16)
        nc.sync.dma_start(out=w_sb[:], in_=w_in)
        iota_sb = const.tile([P, 2 * P], bf16)
        nc.sync.dma_start(out=iota_sb[:], in_=iota_t)
        zero_sb = const.tile([P, cfg.wbatch * D], bf16)
        nc.vector.memset(zero_sb[:], 0.0)
        a_sb = const.tile([P, 1], f32)
        nc.sync.dma_start(out=a_sb[:], in_=a_rep)
        if use_bias:
            b_sb = const.tile([P, D], f32)
            nc.sync.dma_start(out=b_sb[:], in_=b_rep)

        # ---- dst norms from in-degrees: nd = (deg>0) / sqrt(max(deg,1)) ----
        width = 2 * tcore
        dg = work.tile([P, width], i32, tag="dg")
        nc.sync.dma_start(out=dg[:], in_=degi)
        f = work.tile([P, width], f32, tag="f")
        nc.vector.tensor_copy(out=f[:], in_=dg[:])
        m = work.tile([P, width], f32, tag="m")
        nc.vector.tensor_scalar(out=m[:], in0=f[:], scalar1=1.0,
                                scalar2=None, op0=Alu.max)
        r = work.tile([P, width], f32, tag="r")
        nc.vector.reciprocal(out=r[:], in_=m[:])
        s = work.tile([P, width], f32, tag="s")
        nc.scalar.activation(out=s[:], in_=r[:], func=Act.Sqrt)
        z = work.tile([P, width], f32, tag="z")
        nc.vector.tensor_scalar(out=z[:], in0=f[:], scalar1=1.0,
                                scalar2=None, op0=Alu.min)
        nd_sb = const.tile([P, width], f32)
        nc.vector.tensor_tensor(out=nd_sb[:], in0=s[:], in1=z[:], op=Alu.mult)
        and_sb = const.tile([P, width], f32)
        nc.vector.tensor_tensor(out=and_sb[:], in0=nd_sb[:],
                                in1=a_sb[:, :1].to_broadcast([P, width]),
                                op=Alu.mult)

        ic0 = 0
        oc0 = 0
        for jidx, (g, si) in enumerate(jobs_flat):
            job = layouts[g][si]
            k0, kn = job["k0"], job["kn"]
            ncht, rc = job["nch_tot"], job["rcols"]
            icols = ncht * 8

            it = ipool.tile([P, nch_max * 8], i16, tag="gidx")
            nc.sync.dma_start(out=it[:, 0:icols],
                              in_=idx_in[:, ic0:ic0 + icols])
            od = odpool.tile([P, 2 * rc_max], f32, tag="offdeg")
            nc.sync.dma_start(out=od[:, 0:2 * rc],
                              in_=od_in[:, oc0:oc0 + 2 * rc])
            # per-column src norm: ns = 1/sqrt(deg)  (deg>=1 by construction)
            nsr = nspool.tile([P, rc_max], f32, tag="nsr")
            nc.vector.reciprocal(out=nsr[:, :rc], in_=od[:, rc:2 * rc])
            nse = nspool.tile([P, rc_max], f32, tag="nse")
            nc.scalar.activation(out=nse[:, :rc], in_=nsr[:, :rc],
                                 func=Act.Sqrt)

            gt = gpool.tile([P, nch_max, D], bf16, tag="gather")
            for b in range(NBANK):
                L = job["L"][b]
                if L == 0:
                    continue
                nchb = job["nch"][b]
                lob = job["lo"][b]
                ni = nchb * P  # full chunks: every read byte gets written
                nc.gpsimd.dma_gather(
                    out_ap=gt[:, lob:lob + nchb, :],
                    in_ap=featsb[b * BR:(b + 1) * BR, :],
                    idxs_ap=it[:, lob * 8:lob * 8 + ni // 16],
                    num_idxs=ni, num_idxs_reg=ni,
                    elem_size=D, single_packet=False)

            stg = stpool.tile([P, cfg.sg, D], bf16, tag="stg")
            for bat in job["batches"]:
                ki, bn = bat["jb0"], bat["bn"]
                ap_ = ppool.tile([P, cfg.wbatch, D], f32)
                nc.tensor.matmul(out=ap_[:, :bn, :].rearrange("p a b -> p (a b)"),
                                 lhsT=w_sb[:], rhs=zero_sb[:, :bn * D],
                                 start=True, stop=False, skip_group_check=True)
                for (col, c, j2, nt, stop) in bat["plan"]:
                    w_oh = nt * P
                    oh = ohpool.tile([P, 2 * P], bf16)
                    nc.vector.tensor_scalar(
                        out=oh[:, :w_oh], in0=iota_sb[:, :w_oh],
                        scalar1=od[:, col:col + 1],
                        scalar2=nse[:, col:col + 1],
                        op0=Alu.is_equal, op1=Alu.mult)
                    nc.tensor.matmul(
                        out=ap_[:, j2:j2 + nt, :].rearrange("p a b -> p (a b)"),
                        lhsT=gt[:, c, :], rhs=oh[:, :w_oh],
                        start=False, stop=stop, skip_group_check=True)
                asb = apool.tile([P, cfg.wbatch, D], bf16, tag="aggsb")
                nc.scalar.activation(out=asb[:, :bn, :], in_=ap_[:, :bn, :],
                                     func=Act.Copy)
                for j2 in range(bn):
                    h = hpool.tile([P, D], f32)
                    nc.tensor.matmul(out=h[:], lhsT=asb[:, j2, :], rhs=w_sb[:],
                                     start=True, stop=True)
                    kslot = g * tcore + k0 + ki + j2
                    if cfg.act_prelu and not use_bias:
                        nc.scalar.activation(
                            out=stg[:, ki + j2, :], in_=h[:], func=Act.Prelu,
                            scale=nd_sb[:, kslot:kslot + 1], alpha=a_sb[:, :1])
                        continue
                    if use_bias:
                        hb = tpool.tile([P, D], f32, tag="hb")
                        nc.vector.tensor_scalar(
                            out=hb[:], in0=h[:],
                            scalar1=nd_sb[:, kslot:kslot + 1],
                            scalar2=None, op0=Alu.mult)
                        hb2 = tpool.tile([P, D], f32, tag="hb2")
                        nc.vector.tensor_tensor(out=hb2[:], in0=hb[:],
                                                in1=b_sb[:], op=Alu.add)
                        neg = tpool.tile([P, D], f32, tag="neg")
                        nc.vector.tensor_scalar(
                            out=neg[:], in0=hb2[:], scalar1=0.0,
                            scalar2=a_sb[:, :1], op0=Alu.min, op1=Alu.mult)
                        pos = tpool.tile([P, D], f32, tag="pos")
                        nc.vector.tensor_scalar(
                            out=pos[:], in0=hb2[:], scalar1=0.0,
                            scalar2=None, op0=Alu.max)
                    else:
                        neg = tpool.tile([P, D], f32, tag="neg")
                        nc.vector.tensor_scalar(
                            out=neg[:], in0=h[:], scalar1=0.0,
                            scalar2=and_sb[:, kslot:kslot + 1],
                            op0=Alu.min, op1=Alu.mult)
                        pos = tpool.tile([P, D], f32, tag="pos")
                        nc.vector.tensor_scalar(
                            out=pos[:], in0=h[:], scalar1=0.0,
                            scalar2=nd_sb[:, kslot:kslot + 1],
                            op0=Alu.max, op1=Alu.mult)
                    nc.vector.tensor_tensor(out=stg[:, ki + j2, :], in0=neg[:],
                                            in1=pos[:], op=Alu.add)
                ki += bn
            nc.sync.dma_start(out=out[g, :, k0:k0 + kn, :], in_=stg[:, :kn, :])
            ic0 += icols
            oc0 += 2 * rc
    return out


# --------------------------------------------------------------------------
# Driver
# --------------------------------------------------------------------------
def _build_program(cfg: Config, meta):
    import concourse.bacc as bacc
    import concourse.tile as tile

    nc = bacc.Bacc("TRN2", target_bir_lowering=False, debug=False,
                   enable_asserts=False, num_devices=cfg.n_cores)
    with tile.TileContext(nc) as tc:
        build_kernel(nc, tc, cfg, meta)
    nc.compile()
    return nc


def _unscramble(results, plans, cfg: Config):
    n = cfg.n_nodes
    full = np.zeros((2, n, D), np.float32)
    for g in range(2):
        ct_all = plans[g]["core_tiles"]
        for core in range(cfg.n_cores):
            oc = np.asarray(results[core]["out"], dtype=np.float32)
            # oc: [2, P, t_core, D]
            for k in range(cfg.t_core):
                t = int(ct_all[core, k])
                if t < 0:
                    continue
                r0 = t * P
                r1 = min(r0 + P, n)
                full[g, r0:r1] = oc[g, :r1 - r0, k, :]
    return full


_PROGRAM_CACHE = {}


def _meta_key(cfg: Config, meta):
    sig = (cfg.n_nodes, cfg.n_cores, cfg.sg, cfg.wbatch, cfg.gbufs, cfg.ipb,
           cfg.ohb, cfg.act_prelu, meta["use_bias"], meta["jobs_flat"],
           meta["layouts"])
    return hashlib.md5(pickle.dumps(sig)).hexdigest()


def run(inputs, cfg: Config, trace=False):
    from concourse.bass_utils import run_bass_kernel_spmd

    in_maps, plans, meta = preprocess(
        inputs["feats"], inputs["W"], inputs["b"], inputs["prelu_a"],
        inputs["src_pos"], inputs["dst_pos"],
        inputs["src_neg"], inputs["dst_neg"], cfg)

    key = _meta_key(cfg, meta)
    nc = _PROGRAM_CACHE.get(key)
    if nc is None:
        nc = _build_program(cfg, meta)
        _PROGRAM_CACHE[key] = nc

    kwargs = {}
    if trace:
        kwargs = dict(trace=True, tmpdir=tempfile.mkdtemp(prefix="bgc_trace_"))
    res = run_bass_kernel_spmd(nc, in_maps, core_ids=list(range(cfg.n_cores)),
                               **kwargs)
    full = _unscramble(res.results, plans, cfg)
    return full, res


def kernel(**inputs) -> np.ndarray:
    cfg = Config()
    full, _ = run(inputs, cfg)
    return full


# revision 40
# speedup vs baseline: 1.1426x; 1.0264x over previous
"""Bass/Trainium2 kernel for BiGraphContrastLayer (GNN message passing).

Computes, for two edge lists (pos/neg) over the same node features:
    h_g = PReLU( D_in^-1/2 A_g D_out^-1/2 feats @ W + b )
returning stack([h_pos, h_neg]) of shape [2, N, Dout].

Strategy (8 NeuronCores, SPMD, no collectives):

Because row-scaling commutes with the right-multiply by W,
    h = nd . ((segsum(ns[src] * feats[src])) @ W),
we gather FEATS rows directly (no feats@W pre-pass at all), fold the
per-src norm ns into the one-hot segment-sum matrix, aggregate
transposed (aggT[d, i] = sum_slot gt[slot, d] * oh[slot, i], i.e.
matmul(lhsT=gathered_chunk, rhs=onehot)), then apply @W per dst tile
and PReLU with the dst norm as the activation scale.

Work split: dst tiles are dealt to the 8 cores (sorted by edge count so
the shared instruction stream is near-optimal for every core). Edges
are bucketed (dst_tile, src_bank) and packed tightly into 128-slot
gather chunks: slot counts are padded only to the cross-core max per
(tile, bank) -- not to a multiple of 128 -- and the dma_gather calls
use exact num_idxs. Chunks spanning two dst tiles are consumed by both
tiles' one-hot matmuls (the one-hot zeroes foreign edges via a
supergroup-relative offset encoding).

Host does integer index work only (sorting, bucketing, degree
bincounts, int16 gather indices); norms are computed on device from
gathered integer degrees.
"""

import hashlib
import math
import pickle
import tempfile
from dataclasses import dataclass

import numpy as np

P = 128   # partitions
D = 128   # feature dim (Din == Dout == 128)
NBANK = 4  # int16 gather indices -> <=32768 rows per gather window
SENT = 512.0  # one-hot sentinel: never matches iota (0..127)


def _cdiv(a, b):
    return -(-a // b)


# --------------------------------------------------------------------------
# Config
# --------------------------------------------------------------------------
@dataclass
class Config:
    n_nodes: int = 100000
    n_cores: int = 8
    sg: int = 9        # dst-tile positions per supergroup (job)
    wbatch: int = 4    # dst tiles per PSUM bank / W-matmul batch
    gbufs: int = 2     # gather buffer count
    ipb: int = 4       # idx buffer count
    ohb: int = 32      # one-hot buffer count
    odb: int = 4       # off/deg buffer count
    ppb: int = 3       # PSUM agg banks
    hpb: int = 4       # PSUM h tiles
    act_prelu: bool = True   # ScalarE Prelu (not implemented in CoreSim)
    xbatch: int = 8    # unused (kept for test harness compat)

    @property
    def t_global(self) -> int:
        return math.ceil(self.n_nodes / P)

    @property
    def n_pad(self) -> int:
        return self.t_global * P

    @property
    def t_core(self) -> int:
        return math.ceil(self.t_global / self.n_cores)

    @property
    def bank_rows(self) -> int:
        br = _cdiv(_cdiv(self.n_pad, NBANK), P) * P
        assert br <= 32768
        return br

    @property
    def n_pad2(self) -> int:
        return self.bank_rows * NBANK


# --------------------------------------------------------------------------
# Host-side preprocessing (integer index manipulation only)
# --------------------------------------------------------------------------
def _plan_graph(src, dst, cfg: Config):
    """Bucket edges by (dst tile, src bank); deal tiles to cores."""
    tg, ncores, tcore = cfg.t_global, cfg.n_cores, cfg.t_core
    BR = cfg.bank_rows
    src = np.asarray(src, np.int64)
    dst = np.asarray(dst, np.int64)
    deg_out = np.bincount(src, minlength=cfg.n_nodes)
    deg_in = np.bincount(dst, minlength=cfg.n_nodes)

    t = dst // P
    bk = src // BR
    key = t * NBANK + bk
    order = np.argsort(key, kind="stable")
    loc_s = (src - bk * BR)[order].astype(np.int16)
    off_s = (dst[order] % P).astype(np.float32)
    deg_s = deg_out[src[order]].astype(np.int32)
    cnt = np.bincount(key, minlength=tg * NBANK).reshape(tg, NBANK)
    starts = np.zeros(tg * NBANK + 1, np.int64)
    np.cumsum(cnt.reshape(-1), out=starts[1:])

    # Deal tiles to cores: sort by total count desc so each slot's group
    # of n_cores tiles has near-equal size (shared instruction stream),
    # then hill-climb swaps between nearby groups to reduce the
    # per-(group, bank) max (which pads every core's gather).
    tot = cnt.sum(1)
    ordt = np.argsort(-tot, kind="stable")
    nslots = ncores * tcore
    dealt = np.full(nslots, -1, np.int64)
    dealt[:tg] = ordt
    groups = dealt.reshape(tcore, ncores)  # [slot, core] tile ids
    _refine_deal(groups, cnt, cfg)
    _order_slots(groups, cnt, cfg)
    core_tiles = groups.T.copy()  # [ncores, tcore]

    # Shared per-(slot, bank) counts = max over the slot's group.
    C = np.zeros((tcore, NBANK), np.int64)
    for k in range(tcore):
        grp = core_tiles[:, k]
        grp = grp[grp >= 0]
        if len(grp):
            C[k] = cnt[grp].max(0)
        if C[k].sum() == 0:
            C[k, 0] = 1  # keep PSUM chain non-empty for null slots
    return dict(core_tiles=core_tiles, C=C, cnt=cnt, starts=starts,
                loc_s=loc_s, off_s=off_s, deg_s=deg_s,
                deg_out=deg_out, deg_in=deg_in)


def _refine_deal(groups, cnt, cfg: Config, dist=3, sweeps=3):
    """Swap tiles between nearby slot-groups to reduce sum of per-bank
    maxima (= shared gather padding). groups: [t_core, n_cores] in/out."""
    tcore, ncores = groups.shape
    zero = np.zeros(NBANK, np.int64)

    def bank(t):
        return cnt[t] if t >= 0 else zero

    G = np.stack([np.stack([bank(t) for t in groups[k]]) for k in range(tcore)])
    # G: [tcore, ncores, NBANK]
    for _ in range(sweeps):
        improved = False
        for d in range(1, dist + 1):
            for k in range(tcore - d):
                k2 = k + d
                A, B = G[k], G[k2]
                base = A.max(0).sum() + B.max(0).sum()
                best = (0, -1, -1)
                for i in range(ncores):
                    Ai = np.delete(A, i, axis=0)
                    Am = Ai.max(0)
                    for j in range(ncores):
                        Bj = np.delete(B, j, axis=0)
                        c = (np.maximum(Am, B[j]).sum()
                             + np.maximum(Bj.max(0), A[i]).sum())
                        delta = c - base
                        if delta < best[0]:
                            best = (delta, i, j)
                if best[1] >= 0:
                    i, j = best[1], best[2]
                    groups[k, i], groups[k2, j] = groups[k2, j], groups[k, i]
                    G[k, i], G[k2, j] = G[k2, j].copy(), G[k, i].copy()
                    improved = True
        if not improved:
            break


def _order_slots(groups, cnt, cfg: Config):
    """Permute slot order so each supergroup gets a balanced mix of big
    and small tile-groups (keeps the gather DMA stream dense throughout)
    and the final supergroup gets the smallest ones (short drain tail)."""
    tcore = groups.shape[0]
    sizes = np.zeros(tcore, np.int64)
    for k in range(tcore):
        grp = groups[k]
        grp = grp[grp >= 0]
        if len(grp):
            sizes[k] = cnt[grp].max(0).sum()
    njobs = _cdiv(tcore, cfg.sg)
    order = np.argsort(-sizes, kind="stable")
    if njobs <= 1:
        return
    last_kn = tcore - (njobs - 1) * cfg.sg
    head, tail = order[:tcore - last_kn], order[tcore - last_kn:]
    nj = njobs - 1
    perm = []
    for j in range(nj):
        perm.extend(head[j::nj])
    perm.extend(tail)
    groups[:] = groups[np.array(perm)]


def _layout_graph(plan, cfg: Config):
    """Shared job layout: per supergroup, per bank, slot space and the
    per-tile (column, chunk) consumption lists. A chunk spanning two
    tiles gets one masked one-hot column per tile (masking is baked
    into the host-built off values)."""
    C = plan["C"]
    tcore, sg = cfg.t_core, cfg.sg
    jobs = []
    k0 = 0
    while k0 < tcore:
        kn = min(sg, tcore - k0)
        Cj = C[k0:k0 + kn]                       # [kn, NBANK]
        L = Cj.sum(0)                            # slots per bank
        nch = -(-L // P)                         # chunks per bank
        lo = np.zeros(NBANK + 1, np.int64)
        np.cumsum(nch, out=lo[1:])
        pfx = np.zeros((kn + 1, NBANK), np.int64)
        np.cumsum(Cj, axis=0, out=pfx[1:])
        # per-tile chunk ranges
        ranges = [[] for _ in range(kn)]         # ki -> [chunk, ...]
        for ki in range(kn):
            for b in range(NBANK):
                if Cj[ki, b] == 0:
                    continue
                r0 = int(lo[b] + pfx[ki, b] // P)
                r1 = int(lo[b] + _cdiv(int(pfx[ki, b] + Cj[ki, b]), P))
                ranges[ki].extend(range(r0, r1))
        # Merge: a chunk shared by adjacent tiles in the same wbatch batch
        # gets ONE 256-wide one-hot column + ONE matmul into both tiles'
        # PSUM regions (a leading zero-matmul per batch makes start flags
        # moot). cols: (chunk, ki, ntiles); batches: per batch the ordered
        # matmul plan [(colidx, chunk, j2, nt, stop_kis)].
        wb = cfg.wbatch
        cols = []
        batches = []
        for jb0 in range(0, kn, wb):
            bn = min(wb, kn - jb0)
            merged = []
            excl = {ki: [] for ki in range(jb0, jb0 + bn)}
            handled = set()
            for ki in range(jb0, jb0 + bn):
                nxt = ki + 1
                nxt_in = nxt < jb0 + bn
                for c in ranges[ki]:
                    if (c, ki) in handled:
                        continue
                    if (nxt_in and c in ranges[nxt]
                            and len(ranges[ki]) > 1 and len(ranges[nxt]) > 1):
                        ci = len(cols)
                        cols.append((c, ki, 2))
                        merged.append((ci, c, ki - jb0))
                        handled.add((c, ki))
                        handled.add((c, nxt))
                    else:
                        ci = len(cols)
                        cols.append((c, ki, 1))
                        excl[ki].append((ci, c, ki - jb0))
                        handled.add((c, ki))
            # guard: every tile needs >=1 exclusive matmul (stop carrier)
            for ki in range(jb0, jb0 + bn):
                if not excl[ki]:
                    for mi, (cidx, c, j2) in enumerate(merged):
                        kia, kib = cols[cidx][1], cols[cidx][1] + 1
                        if ki in (kia, kib):
                            cols[cidx] = (c, kia, 1)
                            excl[kia].append((cidx, c, kia - jb0))
                            ci2 = len(cols)
                            cols.append((c, kib, 1))
                            excl[kib].append((ci2, c, kib - jb0))
                            merged.pop(mi)
                            break
            plan = []
            for (ci, c, j2) in merged:
                plan.append((ci, c, j2, 2, False))
            for ki in range(jb0, jb0 + bn):
                for n_, (ci, c, j2) in enumerate(excl[ki]):
                    plan.append((ci, c, j2, 1, n_ == len(excl[ki]) - 1))
            batches.append(dict(jb0=jb0, bn=bn, plan=plan))
        coffs = []
        mw = 0
        for (c, ki, nt) in cols:
            coffs.append(mw)
            mw += nt * P
        jobs.append(dict(k0=int(k0), kn=int(kn),
                         L=[int(x) for x in L],
                         nch=[int(x) for x in nch],
                         lo=[int(x) for x in lo[:NBANK]],
                         nch_tot=int(lo[NBANK]),
                         cols=cols, batches=batches, rcols=len(cols),
                         coffs=coffs, mw=int(mw)))
        k0 += kn
    return jobs


def _fill_core_graph(plan, jobs, core, cfg: Config):
    """Per-core idx ([128, cols] int16, x8-replicated wrap) and masked
    per-column off/deg ([128, 2*rcols] bf16) arrays, job-after-job."""
    import ml_dtypes
    bf16 = ml_dtypes.bfloat16
    core_t = plan["core_tiles"][core]
    C, cnt, starts = plan["C"], plan["cnt"], plan["starts"]
    loc_s, off_s, deg_s = plan["loc_s"], plan["off_s"], plan["deg_s"]
    idx_blocks, od_blocks = [], []
    for job in jobs:
        k0, kn, nch_tot = job["k0"], job["kn"], job["nch_tot"]
        nslot = nch_tot * P
        idx_arr = np.zeros(nslot, np.int16)
        own = np.full(nslot, -1, np.int64)
        offv = np.full(nslot, SENT, np.float32)
        degv = np.ones(nslot, np.float32)
        for b in range(NBANK):
            pos = job["lo"][b] * P
            for ki in range(kn):
                cki = int(C[k0 + ki, b])
                if cki == 0:
                    continue
                t = int(core_t[k0 + ki])
                ne = int(cnt[t, b]) if t >= 0 else 0
                if ne:
                    s0 = int(starts[t * NBANK + b])
                    idx_arr[pos:pos + ne] = loc_s[s0:s0 + ne]
                    offv[pos:pos + ne] = off_s[s0:s0 + ne]
                    degv[pos:pos + ne] = deg_s[s0:s0 + ne]
                own[pos:pos + cki] = k0 + ki
                pos += cki
        # wrap [i%16, i//16], replicated to all 128 partitions (the 8 Q7
        # cores each read their own 16-partition stripe on real HW)
        idx_blocks.append(np.tile(idx_arr.reshape(-1, 16).T, (8, 1)))
        rc = job["rcols"]
        offc = np.full((P, rc), SENT, np.float32)
        degc = np.ones((P, rc), np.float32)
        o2 = offv.reshape(nch_tot, P)
        d2 = degv.reshape(nch_tot, P)
        w2 = own.reshape(nch_tot, P)
        for col, (c, ki, nt) in enumerate(job["cols"]):
            m0 = w2[c] == (k0 + ki)
            offc[:, col] = np.where(m0, o2[c], SENT)
            degc[:, col] = np.where(m0, d2[c], 1.0)
            if nt == 2:
                m1 = w2[c] == (k0 + ki + 1)
                offc[:, col] = np.where(m1, o2[c] + P, offc[:, col])
                degc[:, col] = np.where(m1, d2[c], degc[:, col])
        od_blocks.append(np.concatenate([offc, degc], axis=1))
    return np.concatenate(idx_blocks, axis=1), np.concatenate(od_blocks, axis=1)


def preprocess(feats, W, b, prelu_a, src_pos, dst_pos, src_neg, dst_neg,
               cfg: Config):
    import ml_dtypes
    bf16 = ml_dtypes.bfloat16
    n, ncores, tcore = cfg.n_nodes, cfg.n_cores, cfg.t_core
    feats = np.asarray(feats, np.float32)
    W = np.asarray(W, np.float32)
    b = np.asarray(b, np.float32)
    prelu_a = np.asarray(prelu_a, np.float32)

    featsb = np.zeros((cfg.n_pad2, D), bf16)
    featsb[:n] = feats.astype(bf16)

    plans, layouts = [], []
    for src, dst in ((src_pos, dst_pos), (src_neg, dst_neg)):
        plan = _plan_graph(src, dst, cfg)
        plans.append(plan)
        layouts.append(_layout_graph(plan, cfg))

    # interleave the two graphs' jobs
    jobs_flat = []
    for si in range(max(len(layouts[0]), len(layouts[1]))):
        for g in range(2):
            if si < len(layouts[g]):
                jobs_flat.append((g, si))

    degi_arr = np.zeros((ncores, P, 2 * tcore), np.int32)
    for g in range(2):
        dpad = np.zeros(cfg.n_pad, np.int32)
        dpad[:n] = plans[g]["deg_in"]
        dpad_t = dpad.reshape(cfg.t_global, P).T
        for core in range(ncores):
            ct = plans[g]["core_tiles"][core]
            valid = ct >= 0
            degi_arr[core, :, g * tcore:(g + 1) * tcore][:, valid] = (
                dpad_t[:, ct[valid]])

    a_rep = np.full((P, 1), float(prelu_a.reshape(-1)[0]), np.float32)
    w_b = W.astype(bf16)
    b_rep = np.tile(b.reshape(1, D), (P, 1)).astype(np.float32)

    iota_t = np.tile(np.arange(2 * P, dtype=np.float32), (P, 1)).astype(bf16)

    in_maps = []
    for core in range(ncores):
        iw = [None, None]
        od = [None, None]
        for g in range(2):
            iw[g], od[g] = _fill_core_graph(plans[g], layouts[g], core, cfg)
        # assemble in jobs_flat order
        icols = [[0], [0]]
        ocols = [[0], [0]]
        for g in range(2):
            for job in layouts[g]:
                icols[g].append(icols[g][-1] + job["nch_tot"] * 8)
                ocols[g].append(ocols[g][-1] + 2 * job["rcols"])
        idx_parts, od_parts = [], []
        for (g, si) in jobs_flat:
            idx_parts.append(iw[g][:, icols[g][si]:icols[g][si + 1]])
            od_parts.append(od[g][:, ocols[g][si]:ocols[g][si + 1]])
        in_maps.append({
            "featsb": featsb,
            "w_in": w_b,
            "a_rep": a_rep,
            "b_rep": b_rep,
            "degi": degi_arr[core],
            "idx_in": np.ascontiguousarray(np.concatenate(idx_parts, axis=1)),
            "od_in": np.ascontiguousarray(np.concatenate(od_parts, axis=1)),
            "iota_t": iota_t,
        })
    meta = {
        "layouts": layouts,
        "jobs_flat": jobs_flat,
        "use_bias": bool(np.any(b != 0.0)),
    }
    return in_maps, plans, meta


# --------------------------------------------------------------------------
# Device kernel builder
# --------------------------------------------------------------------------
def build_kernel(nc, tc, cfg: Config, meta):
    from contextlib import ExitStack

    import concourse.mybir as mybir

    f32 = mybir.dt.float32
    bf16 = mybir.dt.bfloat16
    i32 = mybir.dt.int32
    i16 = mybir.dt.int16
    Alu = mybir.AluOpType
    Act = mybir.ActivationFunctionType

    tcore, BR = cfg.t_core, cfg.bank_rows
    layouts = meta["layouts"]
    jobs_flat = meta["jobs_flat"]
    use_bias = meta["use_bias"]

    jobs = [layouts[g][si] for (g, si) in jobs_flat]
    icols_tot = sum(j["nch_tot"] * 8 for j in jobs)
    odcols_tot = sum(2 * j["rcols"] for j in jobs)
    nch_max = max(j["nch_tot"] for j in jobs)
    rc_max = max(j["rcols"] for j in jobs)
    mw_max = max(j["mw"] for j in jobs)

    featsb = nc.dram_tensor("featsb", [cfg.n_pad2, D], bf16,
                            kind="ExternalInput").ap()
    w_in = nc.dram_tensor("w_in", [P, D], bf16, kind="ExternalInput").ap()
    a_rep = nc.dram_tensor("a_rep", [P, 1], f32, kind="ExternalInput").ap()
    b_rep = nc.dram_tensor("b_rep", [P, D], f32, kind="ExternalInput").ap()
    degi = nc.dram_tensor("degi", [P, 2 * tcore], i32, kind="ExternalInput").ap()
    idx_in = nc.dram_tensor("idx_in", [P, icols_tot], i16,
                            kind="ExternalInput").ap()
    od_in = nc.dram_tensor("od_in", [P, odcols_tot], f32,
                           kind="ExternalInput").ap()
    iota_t = nc.dram_tensor("iota_t", [P, 2 * P], bf16,
                            kind="ExternalInput").ap()
    out = nc.dram_tensor("out", [2, P, tcore, D], bf16,
                         kind="ExternalOutput").ap()

    with ExitStack() as ctx:
        const = ctx.enter_context(tc.tile_pool(name="const", bufs=1))
        work = ctx.enter_context(tc.tile_pool(name="work", bufs=2))
        ipool = ctx.enter_context(tc.tile_pool(name="ipool", bufs=cfg.ipb))
        odpool = ctx.enter_context(tc.tile_pool(name="odpool", bufs=cfg.odb))
        nspool = ctx.enter_context(tc.tile_pool(name="nspool", bufs=cfg.odb))
        gpool = ctx.enter_context(tc.tile_pool(name="gpool", bufs=cfg.gbufs))
        ohpool = ctx.enter_context(tc.tile_pool(name="ohpool", bufs=cfg.ohb))
        mpool = ctx.enter_context(tc.tile_pool(name="mpool", bufs=2))
        apool = ctx.enter_context(tc.tile_pool(name="apool", bufs=3))
        stpool = ctx.enter_context(tc.tile_pool(name="stpool", bufs=2))
        tpool = ctx.enter_context(tc.tile_pool(name="tpool", bufs=4))
        ppool = ctx.enter_context(tc.tile_pool(name="ppool", bufs=cfg.ppb,
                                               space="PSUM"))
        hpool = ctx.enter_context(tc.tile_pool(name="hpool", bufs=cfg.hpb,
                                               space="PSUM"))

        # ---- constants ----
        w_sb = const.tile([P, D], bf16)
        nc.sync.dma_start(out=w_sb[:], in_=w_in)
        iota_sb = const.tile([P, 2 * P], bf16)
        nc.sync.dma_start(out=iota_sb[:], in_=iota_t)
        zero_sb = const.tile([P, cfg.wbatch * D], bf16)
        nc.vector.memset(zero_sb[:], 0.0)
        a_sb = const.tile([P, 1], f32)
        nc.sync.dma_start(out=a_sb[:], in_=a_rep)
        if use_bias:
            b_sb = const.tile([P, D], f32)
            nc.sync.dma_start(out=b_sb[:], in_=b_rep)

        # ---- dst norms from in-degrees: nd = (deg>0) / sqrt(max(deg,1)) ----
        width = 2 * tcore
        dg = work.tile([P, width], i32, tag="dg")
        nc.sync.dma_start(out=dg[:], in_=degi)
        f = work.tile([P, width], f32, tag="f")
        nc.vector.tensor_copy(out=f[:], in_=dg[:])
        m = work.tile([P, width], f32, tag="m")
        nc.vector.tensor_scalar(out=m[:], in0=f[:], scalar1=1.0,
                                scalar2=None, op0=Alu.max)
        r = work.tile([P, width], f32, tag="r")
        nc.vector.reciprocal(out=r[:], in_=m[:])
        s = work.tile([P, width], f32, tag="s")
        nc.scalar.activation(out=s[:], in_=r[:], func=Act.Sqrt)
        z = work.tile([P, width], f32, tag="z")
        nc.vector.tensor_scalar(out=z[:], in0=f[:], scalar1=1.0,
                                scalar2=None, op0=Alu.min)
        nd_sb = const.tile([P, width], f32)
        nc.vector.tensor_tensor(out=nd_sb[:], in0=s[:], in1=z[:], op=Alu.mult)
        and_sb = const.tile([P, width], f32)
        nc.vector.tensor_tensor(out=and_sb[:], in0=nd_sb[:],
                                in1=a_sb[:, :1].to_broadcast([P, width]),
                                op=Alu.mult)

        ic0 = 0
        oc0 = 0
        for jidx, (g, si) in enumerate(jobs_flat):
            job = layouts[g][si]
            k0, kn = job["k0"], job["kn"]
            ncht, rc = job["nch_tot"], job["rcols"]
            icols = ncht * 8

            it = ipool.tile([P, nch_max * 8], i16, tag="gidx")
            nc.sync.dma_start(out=it[:, 0:icols],
                              in_=idx_in[:, ic0:ic0 + icols])
            od = odpool.tile([P, 2 * rc_max], f32, tag="offdeg")
            nc.sync.dma_start(out=od[:, 0:2 * rc],
                              in_=od_in[:, oc0:oc0 + 2 * rc])
            # per-column src norm: ns = 1/sqrt(deg)  (deg>=1 by construction)
            nsr = nspool.tile([P, rc_max], f32, tag="nsr")
            nc.vector.reciprocal(out=nsr[:, :rc], in_=od[:, rc:2 * rc])
            nse = nspool.tile([P, rc_max], f32, tag="nse")
            nc.scalar.activation(out=nse[:, :rc], in_=nsr[:, :rc],
                                 func=Act.Sqrt)

            gt = gpool.tile([P, nch_max, D], bf16, tag="gather")
            for b in range(NBANK):
                L = job["L"][b]
                if L == 0:
                    continue
                nchb = job["nch"][b]
                lob = job["lo"][b]
                ni = nchb * P  # full chunks: every read byte gets written
                nc.gpsimd.dma_gather(
                    out_ap=gt[:, lob:lob + nchb, :],
                    in_ap=featsb[b * BR:(b + 1) * BR, :],
                    idxs_ap=it[:, lob * 8:lob * 8 + ni // 16],
                    num_idxs=ni, num_idxs_reg=ni,
                    elem_size=D, single_packet=False)

            stg = stpool.tile([P, cfg.sg, D], bf16, tag="stg")
            mt = mpool.tile([P, mw_max], bf16, tag="ohmega")
            coffs = job["coffs"]
            for bat in job["batches"]:
                ki, bn = bat["jb0"], bat["bn"]
                ap_ = ppool.tile([P, cfg.wbatch, D], f32)
                nc.tensor.matmul(out=ap_[:, :bn, :].rearrange("p a b -> p (a b)"),
                                 lhsT=w_sb[:], rhs=zero_sb[:, :bn * D],
                                 start=True, stop=False, skip_group_check=True)
                for (col, c, j2, nt, stop) in bat["plan"]:
                    w_oh = nt * P
                    co = coffs[col]
                    nc.vector.tensor_scalar(
                        out=mt[:, co:co + w_oh], in0=iota_sb[:, :w_oh],
                        scalar1=od[:, col:col + 1],
                        scalar2=nse[:, col:col + 1],
                        op0=Alu.is_equal, op1=Alu.mult)
                    nc.tensor.matmul(
                        out=ap_[:, j2:j2 + nt, :].rearrange("p a b -> p (a b)"),
                        lhsT=gt[:, c, :], rhs=mt[:, co:co + w_oh],
                        start=False, stop=stop, skip_group_check=True)
                asb = apool.tile([P, cfg.wbatch, D], bf16, tag="aggsb")
                nc.scalar.activation(out=asb[:, :bn, :], in_=ap_[:, :bn, :],
                                     func=Act.Copy)
                for j2 in range(bn):
                    h = hpool.tile([P, D], f32)
                    nc.tensor.matmul(out=h[:], lhsT=asb[:, j2, :], rhs=w_sb[:],
                                     start=True, stop=True)
                    kslot = g * tcore + k0 + ki + j2
                    if cfg.act_prelu and not use_bias:
                        nc.scalar.activation(
                            out=stg[:, ki + j2, :], in_=h[:], func=Act.Prelu,
                            scale=nd_sb[:, kslot:kslot + 1], alpha=a_sb[:, :1])
                        continue
                    if use_bias:
                        hb = tpool.tile([P, D], f32, tag="hb")
                        nc.vector.tensor_scalar(
                            out=hb[:], in0=h[:],
                            scalar1=nd_sb[:, kslot:kslot + 1],
                            scalar2=None, op0=Alu.mult)
                        hb2 = tpool.tile([P, D], f32, tag="hb2")
                        nc.vector.tensor_tensor(out=hb2[:], in0=hb[:],
                                                in1=b_sb[:], op=Alu.add)
                        neg = tpool.tile([P, D], f32, tag="neg")
                        nc.vector.tensor_scalar(
                            out=neg[:], in0=hb2[:], scalar1=0.0,
                            scalar2=a_sb[:, :1], op0=Alu.min, op1=Alu.mult)
                        pos = tpool.tile([P, D], f32, tag="pos")
                        nc.vector.tensor_scalar(
                            out=pos[:], in0=hb2[:], scalar1=0.0,
                            scalar2=None, op0=Alu.max)
                    else:
                        neg = tpool.tile([P, D], f32, tag="neg")
                        nc.vector.tensor_scalar(
                            out=neg[:], in0=h[:], scalar1=0.0,
                            scalar2=and_sb[:, kslot:kslot + 1],
                            op0=Alu.min, op1=Alu.mult)
                        pos = tpool.tile([P, D], f32, tag="pos")
                        nc.vector.tensor_scalar(
                            out=pos[:], in0=h[:], scalar1=0.0,
                            scalar2=nd_sb[:, kslot:kslot + 1],
                            op0=Alu.max, op1=Alu.mult)
                    nc.vector.tensor_tensor(out=stg[:, ki + j2, :], in0=neg[:],
                                            in1=pos[:], op=Alu.add)
                ki += bn
            nc.sync.dma_start(out=out[g, :, k0:k0 + kn, :], in_=stg[:, :kn, :])
            ic0 += icols
            oc0 += 2 * rc
    return out


# --------------------------------------------------------------------------
# Driver
# --------------------------------------------------------------------------
def _build_program(cfg: Config, meta):
    import concourse.bacc as bacc
    import concourse.tile as tile

    nc = bacc.Bacc("TRN2", target_bir_lowering=False, debug=False,
                   enable_asserts=False, num_devices=cfg.n_cores)
    with tile.TileContext(nc) as tc:
        build_kernel(nc, tc, cfg, meta)
    nc.compile()
    return nc


def _unscramble(results, plans, cfg: Config):
    n = cfg.n_nodes
    full = np.zeros((2, n, D), np.float32)
    for g in range(2):
        ct_all = plans[g]["core_tiles"]
        for core in range(cfg.n_cores):
            oc = np.asarray(results[core]["out"], dtype=np.float32)
            # oc: [2, P, t_core, D]
            for k in range(cfg.t_core):
                t = int(ct_all[core, k])
                if t < 0:
                    continue
                r0 = t * P
                r1 = min(r0 + P, n)
                full[g, r0:r1] = oc[g, :r1 - r0, k, :]
    return full


_PROGRAM_CACHE = {}


def _meta_key(cfg: Config, meta):
    sig = (cfg.n_nodes, cfg.n_cores, cfg.sg, cfg.wbatch, cfg.gbufs, cfg.ipb,
           cfg.ohb, cfg.odb, cfg.ppb, cfg.hpb, cfg.act_prelu,
           meta["use_bias"], meta["jobs_flat"], meta["layouts"])
    return hashlib.md5(pickle.dumps(sig)).hexdigest()


def run(inputs, cfg: Config, trace=False):
    from concourse.bass_utils import run_bass_kernel_spmd

    in_maps, plans, meta = preprocess(
        inputs["feats"], inputs["W"], inputs["b"], inputs["prelu_a"],
        inputs["src_pos"], inputs["dst_pos"],
        inputs["src_neg"], inputs["dst_neg"], cfg)

    key = _meta_key(cfg, meta)
    nc = _PROGRAM_CACHE.get(key)
    if nc is None:
        nc = _build_program(cfg, meta)
        _PROGRAM_CACHE[key] = nc

    kwargs = {}
    if trace:
        kwargs = dict(trace=True, tmpdir=tempfile.mkdtemp(prefix="bgc_trace_"))
    res = run_bass_kernel_spmd(nc, in_maps, core_ids=list(range(cfg.n_cores)),
                               **kwargs)
    full = _unscramble(res.results, plans, cfg)
    return full, res


def kernel(**inputs) -> np.ndarray:
    cfg = Config()
    full, _ = run(inputs, cfg)
    return full
